# revision 1
# baseline (speedup 1.0000x reference)
"""Multi-head attention (B=4, N=2048, C=1024, H=16, Dh=64) on 8 TRN2 NeuronCores.

Sharding: tensor-parallel over heads — core c owns heads (2c, 2c+1) for all
batches.  Each core computes its 2 heads' QKV projection, attention, and the
partial output projection (contraction over its 128 head-dims of w_proj);
the host sums the 8 partial projections and adds the bias.

Per-core pipeline (unit = one batch of 2048 tokens):
  - host passes xT = x^T [1024, 8192] so channels land on SBUF partitions
  - QT/KT/VT computed as [128(d, 2 heads stacked), t] tiles
  - scores computed TRANSPOSED: ST[k, q] = KT_h.T @ QT_h (contraction d=64,
    two heads row-packed into the PE array: h0 rows 0-63, h1 rows 64-127,
    running concurrently via tile_position row groups)
  - softmax without max-subtraction (scores verified: |s|*scale < 10):
    ACT exp reads the score PSUM pair [128, 1024] directly, writes PT
  - AV: O^T[d, q] with lhsT = [V_h | ones] (M=65): PSUM row 64 accumulates
    the softmax denominator for free; the accumulator is evicted to SBUF
    immediately (frees the PSUM bank) and normalized off the critical path
  - proj: out[t, o] = OT_tile.T @ wpT, evicted right after each q-span

Scheduling: the kernel is a software pipeline clocked by the Scalar
engine's exp (~1.06us per k-chunk).  Attention is emitted in 2-k-chunk
super-slots: both score pairs back-to-back (the second pair's LDWEIGHTS
hides under the first — K=64 row-group reuse), then a cost-budgeted
amount of filler (next unit's QKV / V-transposes / previous q-span's
proj, at SINGLE-matmul granularity), then the AV pairs of the PREVIOUS
super-slot (staggered so the in-order PE queue never waits on ACT).

MHA_DTYPE env: "bf16" (default) or "f32r" or "f32" — matmul input dtype.
PSUM accumulation and softmax statistics are always fp32.
"""

import os
import numpy as np

B, N, C = 4, 2048, 1024
H, Dh = 16, 64
NT = B * N            # 8192 tokens
NCORES = 8
HPC = H // NCORES     # 2 heads per core
SCALE = Dh ** -0.5

TPU = N               # tokens per unit (one batch)
QS = 512              # q-span
KC = 128              # k-chunk
MHA_DTYPE = os.environ.get("MHA_DTYPE", "bf16")
FILL_BUDGET = float(os.environ.get("MHA_FILL", "4.6"))
PAIRK = int(os.environ.get("MHA_PAIRK", "2"))      # k-chunks per super-slot
WSPLIT = os.environ.get("MHA_WSPLIT", "0") == "1"  # per-cc weight tiles
NPOP2 = os.environ.get("MHA_NPOP2", "1") == "1"    # early qkv pops
DEFER = os.environ.get("MHA_DEFER", "0") == "1"    # defer proj into tail unit
WARMUP_MMS = int(os.environ.get("MHA_WARMUP", "16"))
PREPUSH = os.environ.get("MHA_PREPUSH", "0") == "1"  # fill startup slots

_CACHE = {}


def _np_in_dtype():
    if MHA_DTYPE == "bf16":
        import ml_dtypes
        return np.dtype(ml_dtypes.bfloat16)
    return np.dtype(np.float32)


def _build_program():
    import concourse.bacc as bacc
    import concourse.bass as bass
    import concourse.tile as tile
    from concourse import mybir
    from concourse.masks import make_identity

    f32 = mybir.dt.float32
    din = {
        "bf16": mybir.dt.bfloat16,
        "f32r": mybir.dt.float32r,
        "f32": mybir.dt.float32,
    }[MHA_DTYPE]

    nc = bacc.Bacc("TRN2", target_bir_lowering=False, debug=False)

    xT = nc.dram_tensor("xT", [C, NT], din, kind="ExternalInput").ap()
    wqkvT = nc.dram_tensor("wqkvT", [C, 6 * Dh], din, kind="ExternalInput").ap()
    wpT = nc.dram_tensor("wpT", [2 * Dh, C], din, kind="ExternalInput").ap()
    out = nc.dram_tensor("out", [NT, C], f32, kind="ExternalOutput").ap()

    NCC = C // 128        # 8 c-chunks
    NTT = TPU // QS       # 4 t-tiles per unit
    NKC = TPU // KC       # 16 k-chunks per unit
    NQS = TPU // QS       # 4 q-spans per unit
    KPT = QS // KC        # 4 k-chunks per t-tile
    VW = 2 * (Dh + 1)     # 130: V_sb row layout [V_h0 | 1 | V_h1 | 1]

    with tile.TileContext(nc) as tc:
        with (
            tc.tile_pool(name="const", bufs=1) as const,
            tc.tile_pool(name="xp", bufs=32) as xp,
            tc.tile_pool(name="qt", bufs=2) as qtp,
            tc.tile_pool(name="kt", bufs=2) as ktp,
            tc.tile_pool(name="vt", bufs=2) as vtp,
            tc.tile_pool(name="vsb", bufs=2) as vsbp,
            tc.tile_pool(name="pt", bufs=4) as ptp,
            tc.tile_pool(name="ot", bufs=4 if DEFER else 2) as otp,
            tc.tile_pool(name="rn", bufs=2) as rnp,
            tc.tile_pool(name="po", bufs=4) as pop,
            tc.tile_pool(name="mps", bufs=2, space="PSUM") as mps,
            tc.tile_pool(name="stps", bufs=2, space="PSUM") as stps,
            tc.tile_pool(name="avps", bufs=1, space="PSUM") as avps,
        ):
            ident = const.tile([128, 128], din)
            make_identity(nc, ident)

            # ramp: pull the ACT exp-table load (~2.7us) off the critical
            # path, and keep the PE busy during the initial x/w DMAs so the
            # HAM clock-gate is released before the first real matmul
            def warm_mm():
                # real matmul (transpose-mode does NOT engage the HAM
                # clock-gate): keeps/brings the PE at 2.4 GHz
                wps = mps.tile([128, 128], f32, tag="m", name="wps")
                nc.tensor.matmul(wps, ident, ident, skip_group_check=True)

            if WARMUP_MMS > 0:
                warm_out = const.tile([128, 8], f32)
                nc.scalar.activation(
                    warm_out, ident[:, 0:8],
                    mybir.ActivationFunctionType.Exp)
                for _ in range(WARMUP_MMS):
                    warm_mm()

            # per-cc weight tiles: first QKV matmul only waits on its own chunk
            wq_sbs = []
            if WSPLIT:
                for cc in range(NCC):
                    w_t = const.tile([128, 6 * Dh], din)
                    nc.gpsimd.dma_start(
                        out=w_t,
                        in_=wqkvT[cc * 128:(cc + 1) * 128, :],
                    )
                    wq_sbs.append(w_t)
            else:
                wq_sb = const.tile([128, NCC * 6 * Dh], din)
                for cc in range(NCC):
                    nc.gpsimd.dma_start(
                        out=wq_sb[:, cc * 6 * Dh:(cc + 1) * 6 * Dh],
                        in_=wqkvT[cc * 128:(cc + 1) * 128, :],
                    )
                    wq_sbs.append(wq_sb[:, cc * 6 * Dh:(cc + 1) * 6 * Dh])
            wp_sb = const.tile([128, C], din)
            nc.gpsimd.dma_start(out=wp_sb, in_=wpT)

            # per-unit persistent tiles, allocated lazily
            QT, KT, VT, VSB, OT = {}, {}, {}, {}, {}

            def alloc_unit(u):
                QT[u] = qtp.tile([128, TPU], din, tag="QT", name=f"QT{u}")
                KT[u] = ktp.tile([128, TPU], din, tag="KT", name=f"KT{u}")
                VT[u] = vtp.tile([128, TPU], din, tag="VT", name=f"VT{u}")
                VSB[u] = vsbp.tile([128, NKC * VW], din, tag="VSB", name=f"VSB{u}")

            def qkv_items(u, tt, early_load=False):
                """QKV projection + V transposes for t-tile tt of unit u, as
                (cost, closure) items at single-matmul granularity.  With
                early_load the x DMAs are issued immediately (at block top,
                ~16 slots before the matmuls drain) so their latency is
                hidden instead of stalling the first matmul of the group."""
                items = []
                state = {}

                def load_x():
                    if tt == 0:
                        alloc_unit(u)
                    t0 = u * TPU
                    cells = []
                    for cc in range(NCC):
                        xt = xp.tile([128, QS], din, tag="xs", name="xt")
                        nc.sync.dma_start(
                            out=xt,
                            in_=xT[cc * 128:(cc + 1) * 128,
                                   t0 + tt * QS:t0 + (tt + 1) * QS],
                        )
                        cells.append(xt)
                    state["xs"] = cells

                if early_load:
                    load_x()
                else:
                    items.append((0.0, load_x))

                def qkv_mm(grp, cc):
                    def run():
                        if cc == 0:
                            state[grp] = mps.tile([128, QS], f32, tag="m",
                                                  name="ps")
                        ps = state[grp]
                        w_sl = wq_sbs[cc][:, grp * 128:(grp + 1) * 128]
                        nc.tensor.matmul(
                            ps, w_sl, state["xs"][cc],
                            start=(cc == 0), stop=(cc == NCC - 1),
                            skip_group_check=True,
                        )
                        if cc == NCC - 1:
                            tgt = (QT, KT, VT)[grp][u]
                            nc.vector.tensor_copy(
                                tgt[:, tt * QS:(tt + 1) * QS], ps)
                    return run

                for grp in range(3):
                    for cc in range(NCC):
                        items.append((1.0, qkv_mm(grp, cc)))

                def transpose_item(j):
                    def run():
                        kc = tt * KPT + j
                        tp = mps.tile([128, 128], din, tag="m", name="tp")
                        nc.tensor.transpose(
                            tp, VT[u][:, kc * 128:(kc + 1) * 128], ident)
                        base = kc * VW
                        nc.vector.tensor_copy(
                            VSB[u][:, base: base + Dh], tp[:, 0:Dh])
                        nc.vector.memset(
                            VSB[u][:, base + Dh: base + Dh + 1], 1.0)
                        nc.vector.tensor_copy(
                            VSB[u][:, base + Dh + 1: base + 2 * Dh + 1],
                            tp[:, Dh: 2 * Dh])
                        nc.vector.memset(
                            VSB[u][:, base + 2 * Dh + 1: base + VW], 1.0)
                    return run

                for j in range(KPT):
                    items.append((0.8, transpose_item(j)))
                return items

            hard_items = []   # qkv work: must drain before its unit's attn
            soft_items = []   # normalize/proj: anytime
            carry = [0.0]     # fractional pump budget carry

            def pump_budget(budget):
                """Pop items worth ~budget matmul-equivalents of PE time."""
                c = carry[0] + budget
                while c > 0 and (hard_items or soft_items):
                    cost, fn = (hard_items if hard_items else soft_items).pop(0)
                    fn()
                    c -= cost if cost > 0 else 0.0
                    if cost == 0.0:
                        continue
                carry[0] = min(c, 0.0) if (hard_items or soft_items) else 0.0

            def pump_all_hard():
                while hard_items:
                    _, fn = hard_items.pop(0)
                    fn()

            def pump_all():
                pump_all_hard()
                while soft_items:
                    _, fn = soft_items.pop(0)
                    fn()

            # ---- attention: 2-k-chunk super-slots, AVs staggered one
            # super-slot behind their exp so PE never stalls on ACT; score
            # pairs of adjacent k-chunks run back-to-back so the second
            # pair's LDWEIGHTS hides under the first (K=64 row-group reuse)
            av_pending = [[]]

            def emit_scores_exp(u, qs, kc):
                q0 = qs * QS
                sp = stps.tile([128, 2 * QS], f32, name="sp")
                nc.tensor.matmul(
                    sp[:, 0:QS],
                    KT[u][0:Dh, kc * 128:(kc + 1) * 128],
                    QT[u][0:Dh, q0:q0 + QS],
                    skip_group_check=True,
                )
                nc.tensor.matmul(
                    sp[:, QS:2 * QS],
                    KT[u][Dh:128, kc * 128:(kc + 1) * 128],
                    QT[u][Dh:128, q0:q0 + QS],
                    skip_group_check=True,
                )
                pt = ptp.tile([128, 2 * QS], din, name="pt")
                nc.scalar.activation(
                    pt, sp, mybir.ActivationFunctionType.Exp,
                    scale=SCALE,
                )
                return pt

            def make_av(u, oh, kc, pt):
                def run():
                    for i in range(2):
                        vbase = kc * VW + i * (Dh + 1)
                        nc.tensor.matmul(
                            oh[i],
                            VSB[u][:, vbase: vbase + Dh + 1],
                            pt[:, i * QS:(i + 1) * QS],
                            start=(kc == 0), stop=(kc == NKC - 1),
                            skip_group_check=True,
                        )
                return run

            def attn_slots(u, qs, oh, kcs, budget):
                """Super-slots over pairs of k-chunks of q-span qs, unit u.
                Emits scores+exp for (kc0, kc1), pumps filler, then the AV
                pairs of the previous super-slot (stagger behind ACT)."""
                kcs = list(kcs)
                for j in range(0, len(kcs), PAIRK):
                    pair = kcs[j:j + PAIRK]
                    pts = [emit_scores_exp(u, qs, kc) for kc in pair]
                    if budget:
                        pump_budget(budget)
                    for av in av_pending[0]:
                        av()
                    av_pending[0] = [make_av(u, oh, kc, pt)
                                     for kc, pt in zip(pair, pts)]

            def flush_av():
                for av in av_pending[0]:
                    av()
                av_pending[0] = []

            def evict_oh(u, qs, oh):
                """Evict AV accumulators to SBUF (frees PSUM)."""
                flush_av()
                osbs = []
                for i in range(2):
                    osb = rnp.tile([Dh + 1, QS], f32, tag=f"osb{i}",
                                   name=f"osb{i}")
                    nc.vector.tensor_copy(osb, oh[i])
                    osbs.append(osb)
                return osbs

            def normalize_items(u, qs, osbs, c0=0, c1=QS, warm=False):
                if qs == 0 and c0 == 0:
                    OT[u] = otp.tile([128, TPU], din, tag="OT",
                                     name=f"OT{u}")
                q0 = qs * QS
                W = c1 - c0

                def norm(i):
                    def run():
                        osb = osbs[i]
                        d_row = rnp.tile([1, W], f32, tag=f"d{W}",
                                         name="d_row")
                        nc.vector.tensor_copy(d_row, osb[Dh:Dh + 1, c0:c1])
                        r_row = rnp.tile([1, W], f32, tag=f"r{W}",
                                         name="r_row")
                        nc.vector.reciprocal_approx_fast(r_row, d_row)
                        Rb = rnp.tile([Dh, W], f32, tag=f"R{W}", name="Rb")
                        nc.gpsimd.partition_broadcast(Rb, r_row)
                        if i == 0:
                            nc.vector.tensor_mul(
                                OT[u][0:Dh, q0 + c0:q0 + c1],
                                osb[0:Dh, c0:c1], Rb)
                        else:
                            tmp = rnp.tile([Dh, W], din, tag=f"tmp{W}",
                                           name="tmp")
                            nc.vector.tensor_mul(tmp, osb[0:Dh, c0:c1], Rb)
                            nc.sync.dma_start(
                                out=OT[u][Dh:128, q0 + c0:q0 + c1], in_=tmp)
                    return run

                return [(0.0, norm(0)), (0.0, norm(1))]

            def proj_items(u, qs, tail=False):
                t0 = u * TPU

                def proj(tt, osp):
                    def run():
                        pp = mps.tile([128, QS], f32, tag="m", name="pp")
                        nc.tensor.matmul(
                            pp,
                            OT[u][:, tt * 128:(tt + 1) * 128],
                            wp_sb[:, osp * QS:(osp + 1) * QS],
                            skip_group_check=True,
                        )
                        po = pop.tile([128, QS], f32, name="po")
                        if tail and (tt + osp) % 2:
                            # tail: ACT is idle — split PSUM evictions
                            # between the Scalar and Vector engines
                            nc.scalar.copy(po, pp)
                        else:
                            nc.vector.tensor_copy(po, pp)
                        nc.sync.dma_start(
                            out=out[t0 + tt * 128: t0 + (tt + 1) * 128,
                                    osp * QS:(osp + 1) * QS],
                            in_=po,
                        )
                    return run

                items = []
                for tl in range(QS // 128):
                    tt = qs * (QS // 128) + tl
                    for osp in range(C // QS):
                        items.append((1.0, proj(tt, osp)))
                return items

            # ---- software-pipelined emission ----
            pending_qkv = [(u, tt) for u in range(1, B) for tt in range(NTT)]
            pending_qkv.reverse()

            # unit 0 startup: its own qkv blocks drain before each qs=0
            # k-chunk range becomes available
            oh_prev = [avps.tile([Dh + 1, QS], f32, tag=f"av{i}",
                                 name=f"oh{i}") for i in range(2)]
            for tt in range(NTT):
                hard_items.extend(qkv_items(0, tt))
                pump_all_hard()
                attn_slots(0, 0, oh_prev, range(tt * KPT, (tt + 1) * KPT),
                           budget=0)
            prev_block = (0, 0, evict_oh(0, 0, oh_prev))

            block_idx = 0
            deferred_proj = []   # proj work pushed into the last unit's
                                 # ACT-bound slots (no qkv filler there)
            for u in range(B):
                for qs in range(NQS):
                    if u == 0 and qs == 0:
                        continue  # handled in startup
                    block_idx += 1
                    npop = 2 if (block_idx == 1 and NPOP2) else 1
                    for _ in range(npop):
                        if pending_qkv:
                            nu, ntt = pending_qkv.pop()
                            hard_items.extend(
                                qkv_items(nu, ntt, early_load=True))
                    pu, pqs, posbs = prev_block
                    pitems = proj_items(pu, pqs)
                    if DEFER and pu < B - 1:
                        soft_items.extend(normalize_items(pu, pqs, posbs))
                        soft_items.extend(pitems[:4])
                        deferred_proj.extend(pitems[4:])
                    elif pu == B - 1:
                        # last unit: PE has no qkv filler, so proj stalls
                        # on the norm chain — 256-col chunks halve it
                        for ci, (c0, c1) in enumerate(
                                ((0, QS // 2), (QS // 2, QS))):
                            soft_items.extend(
                                normalize_items(pu, pqs, posbs, c0, c1))
                            soft_items.extend(pitems[ci * 4:(ci + 1) * 4])
                    else:
                        soft_items.extend(normalize_items(pu, pqs, posbs))
                        soft_items.extend(pitems)
                    if u == B - 1:
                        # drip deferred proj into the tail unit's slots
                        take = len(deferred_proj) // (NQS - qs)
                        soft_items.extend(deferred_proj[:take])
                        del deferred_proj[:take]
                    if qs == 0:
                        # hard guarantee: unit u's qkv fully emitted before
                        # its attention (emission order is semantic order)
                        pump_all_hard()
                    oh = [avps.tile([Dh + 1, QS], f32, tag=f"av{i}",
                                    name=f"oh{i}") for i in range(2)]
                    attn_slots(u, qs, oh, range(NKC), budget=FILL_BUDGET)
                    prev_block = (u, qs, evict_oh(u, qs, oh))

            # flush: last block's normalize + proj.  The norm chain idles
            # the PE for ~5us (HAM re-throttles to 1.2 GHz), so keep the
            # clock warm with dummy transposes while DVE/GpSimd normalize;
            # tail proj evictions split across Scalar+Vector engines.
            pu, pqs, posbs = prev_block
            soft_items.extend(deferred_proj)

            pitems = proj_items(pu, pqs, tail=True)
            for ci, (c0, c1) in enumerate(((0, QS // 2), (QS // 2, QS))):
                soft_items.extend(normalize_items(pu, pqs, posbs, c0, c1))
                soft_items.extend((0.0, warm_mm) for _ in range(12))
                soft_items.extend(pitems[ci * 4:(ci + 1) * 4])
            pump_all()

    nc.compile()
    return nc


def _shard_inputs(x, w_qkv, w_proj):
    dt = _np_in_dtype()
    xT = np.ascontiguousarray(x.reshape(NT, C).T).astype(dt)
    in_maps = []
    for c in range(NCORES):
        h0, h1 = HPC * c, HPC * c + 1
        rows = []
        for grp in range(3):          # q, k, v
            for h in (h0, h1):
                rows.append(w_qkv[grp * C + h * Dh: grp * C + (h + 1) * Dh])
        wqkvT_c = np.ascontiguousarray(np.concatenate(rows, 0).T).astype(dt)
        wpT_c = np.ascontiguousarray(
            w_proj[:, 2 * Dh * c: 2 * Dh * (c + 1)].T).astype(dt)
        in_maps.append({"xT": xT, "wqkvT": wqkvT_c, "wpT": wpT_c})
    return in_maps


def kernel(x, w_qkv, w_proj, b_proj, _trace=False, _tmpdir=None):
    from concourse import bass_utils

    if "nc" not in _CACHE:
        _CACHE["nc"] = _build_program()
    nc = _CACHE["nc"]

    in_maps = _shard_inputs(
        np.asarray(x, np.float32),
        np.asarray(w_qkv, np.float32),
        np.asarray(w_proj, np.float32),
    )
    res = bass_utils.run_bass_kernel_spmd(
        nc, in_maps, core_ids=list(range(NCORES)),
        trace=_trace, tmpdir=_tmpdir,
    )
    total = res.results[0]["out"].astype(np.float32)
    for c in range(1, NCORES):
        total += res.results[c]["out"]
    total += np.asarray(b_proj, np.float32)[None, :]
    out = total.reshape(B, N, C)
    if _trace:
        return out, res
    return out

